# revision 7
# baseline (speedup 1.0000x reference)
"""CornerNet Trainium2 kernel.

Math (reference):
  t     = kappa * tanh(sign_param) * (x - th)        # (B, R, D)
  s     = sigmoid(t); m = sigmoid(mask_logit)
  gated = 1 - m*(1-s) = (1-m) + m*s
  z     = prod_d gated                               # (B, R)
  y     = z @ head_w.T + head_b                      # (B,)

Device formulation (available act funcs: sigmoid/tanh/exp/ln — no softplus):
  a  = kappa * tanh(sign_param)
  s  = sigmoid(a*x - a*th)     (ScalarE per-partition input affine: scale=a,
                                bias=-a*th — no separate VectorE pass needed)
  g  = c + m*s                 ( c = 1-m = sigmoid(-mask_logit) )
  gp = g[d] * g[d+128]         (pair-product folds the two partition halves)
  lz = sum_{d-pairs} ln(gp) ; z = exp(lz) ; y = z @ w + b

Sharding: tensor-parallel over rules; each of 8 cores takes RC=64 rules and
computes a partial (B,) head dot; host sums partials and adds head_b.

Layout per core: D=256 on partitions (two 128-halves), batch on the free axis.
Per rule: ScalarE sigmoid (input affine does t), VectorE tensor_scalar
g = s*m + c, VectorE tensor_tensor pair-product, ScalarE Ln, then the
partition sum goes to TensorE: lhsT is a (128, 64) sliding window into a
constant matrix whose only nonzero column (+1) is positioned so rule r's sum
lands in PSUM partition r (float32r keeps the PE at 1 cyc/row).  Sigmoid and
Ln live in different activation table sets (~2.7us per switch), so rules are
processed in blocks of KBLK: all sigmoids for the block first, then all Lns.
"""

import numpy as np
from contextlib import ExitStack

import concourse.bass as bass
import concourse.bacc as bacc
import concourse.mybir as mybir
import concourse.tile as tile
from concourse.bass_utils import run_bass_kernel_spmd

B, D, R = 2048, 256, 512
NCORES = 8
RC = R // NCORES            # 64 rules per core
KBLK = 8                    # rules per sigmoid/ln block
CH = 512                    # matmul free-dim chunk (one PSUM bank)
F32 = mybir.dt.float32
F32R = mybir.dt.float32r
AF = mybir.ActivationFunctionType
OP = mybir.AluOpType

_cache = {}


def _build(reps=1):
    nc = bacc.Bacc(None)
    xT = nc.dram_tensor("xT", [D, B], F32, kind="ExternalInput")
    thT = nc.dram_tensor("thT", [D, RC], F32, kind="ExternalInput")
    sgT = nc.dram_tensor("sgT", [D, RC], F32, kind="ExternalInput")
    mkT = nc.dram_tensor("mkT", [D, RC], F32, kind="ExternalInput")
    lkb = nc.dram_tensor("lkb", [128, 1], F32, kind="ExternalInput")
    wcol = nc.dram_tensor("wcol", [RC, 1], F32, kind="ExternalInput")
    selp = nc.dram_tensor("selp", [128, 2 * RC], F32R, kind="ExternalInput")
    y = nc.dram_tensor("y", [1, B], F32, kind="ExternalOutput")

    with tile.TileContext(nc) as tc, ExitStack() as ctx:
        const = ctx.enter_context(tc.tile_pool(name="const", bufs=1))
        sp = ctx.enter_context(tc.tile_pool(name="sp", bufs=2))
        gp_ = ctx.enter_context(tc.tile_pool(name="gp_", bufs=2))
        gpp = ctx.enter_context(tc.tile_pool(name="gpp", bufs=KBLK + 1))
        lp = ctx.enter_context(tc.tile_pool(name="lp", bufs=2))
        psum = ctx.enter_context(
            tc.tile_pool(name="psum", bufs=1, space=bass.MemorySpace.PSUM)
        )

        # ---------------- constant loads ----------------
        xt = []
        for h in range(2):
            t_ = const.tile([128, B], F32, tag=f"xt{h}")
            nc.gpsimd.dma_start(t_[:], xT[h * 128 : (h + 1) * 128, :])
            xt.append(t_)

        tht, sgt, mkt = [], [], []
        for name, dram, lst in (("th", thT, tht), ("sg", sgT, sgt), ("mk", mkT, mkt)):
            for h in range(2):
                t_ = const.tile([128, RC], F32, tag=f"{name}{h}")
                nc.gpsimd.dma_start(t_[:], dram[h * 128 : (h + 1) * 128, :])
                lst.append(t_)

        lkt = const.tile([128, 1], F32, tag="lkt")
        nc.gpsimd.dma_start(lkt[:], lkb[:])
        selpt = const.tile([128, 2 * RC], F32R, tag="selpt")
        nc.gpsimd.dma_start(selpt[:], selp[:])
        wct = const.tile([RC, 1], F32, tag="wct")
        nc.gpsimd.dma_start(wct[:], wcol[:])

        # ---------------- parameter prep ----------------
        kap = const.tile([128, 1], F32, tag="kap")
        nc.scalar.activation(kap[:], lkt[:], AF.Exp)
        nkap = const.tile([128, 1], F32, tag="nkap")
        nc.vector.tensor_scalar(nkap[:], kap[:], -1.0, None, OP.mult)

        aa, nb2, mm_, cc_ = [], [], [], []
        for h in range(2):
            tnh = const.tile([128, RC], F32, tag=f"tnh{h}")
            nc.scalar.activation(tnh[:], sgt[h][:], AF.Tanh)
            a_h = const.tile([128, RC], F32, tag=f"a{h}")
            nc.vector.tensor_scalar(a_h[:], tnh[:], kap[:], None, OP.mult)
            na_h = const.tile([128, RC], F32, tag=f"na{h}")
            nc.vector.tensor_scalar(na_h[:], tnh[:], nkap[:], None, OP.mult)
            nb2_h = const.tile([128, RC], F32, tag=f"nb2{h}")
            nc.vector.tensor_mul(nb2_h[:], na_h[:], tht[h][:])
            aa.append(a_h)
            nb2.append(nb2_h)
            m_h = const.tile([128, RC], F32, tag=f"m{h}")
            nc.scalar.activation(m_h[:], mkt[h][:], AF.Sigmoid)
            c_h = const.tile([128, RC], F32, tag=f"c{h}")
            nc.scalar.activation(c_h[:], mkt[h][:], AF.Sigmoid, scale=-1.0)
            mm_.append(m_h)
            cc_.append(c_h)

        # ---------------- main loop ----------------
        lz = psum.tile([RC, B], F32, tag="lz")
        for rep in range(reps):
            for blk in range(RC // KBLK):
                gps = []
                for k in range(KBLK):
                    r = blk * KBLK + k
                    s = sp.tile([128, 2 * B], F32, tag="s")
                    for h in range(2):
                        nc.scalar.activation(
                            s[:, h * B : (h + 1) * B],
                            xt[h][:],
                            AF.Sigmoid,
                            bias=nb2[h][:, r : r + 1],
                            scale=aa[h][:, r : r + 1],
                        )
                    g = gp_.tile([128, 2 * B], F32, tag="g")
                    for h in range(2):
                        nc.vector.tensor_scalar(
                            g[:, h * B : (h + 1) * B],
                            s[:, h * B : (h + 1) * B],
                            mm_[h][:, r : r + 1],
                            cc_[h][:, r : r + 1],
                            OP.mult,
                            OP.add,
                        )
                    gpt = gpp.tile([128, B], F32, tag="gpt")
                    nc.vector.tensor_mul(gpt[:], g[:, 0:B], g[:, B : 2 * B])
                    gps.append(gpt)
                for k in range(KBLK):
                    r = blk * KBLK + k
                    L = lp.tile([128, B], F32R, tag="L")
                    nc.scalar.activation(L[:], gps[k][:], AF.Ln)
                    lhsp = selpt[:, RC - r : 2 * RC - r]
                    for c in range(B // CH):
                        nc.tensor.matmul(
                            lz[:, c * CH : (c + 1) * CH],
                            lhsp,
                            L[:, c * CH : (c + 1) * CH],
                            start=(r == 0 and rep == 0),
                            stop=(r == RC - 1 and rep == reps - 1),
                        )

        # ---------------- z = exp(lz), head ----------------
        z_sb = const.tile([RC, B], F32, tag="z")
        nc.scalar.activation(z_sb[:], lz[:], AF.Exp)
        yp = psum.tile([1, B], F32, tag="yp")
        for c in range(B // CH):
            nc.tensor.matmul(
                yp[:, c * CH : (c + 1) * CH],
                wct[:],
                z_sb[:, c * CH : (c + 1) * CH],
                start=True,
                stop=True,
            )
        y_sb = const.tile([1, B], F32, tag="ysb")
        nc.vector.tensor_copy(y_sb[:], yp[:])
        nc.sync.dma_start(y[:], y_sb[:])

    nc.compile()
    return nc


def _get_nc(reps=1):
    key = ("nc", reps)
    if key not in _cache:
        _cache[key] = _build(reps)
    return _cache[key]


def _make_in_maps(inputs):
    x = np.ascontiguousarray(inputs["x"], dtype=np.float32)
    th = np.asarray(inputs["th"], dtype=np.float32)
    sg = np.asarray(inputs["sign_param"], dtype=np.float32)
    mk = np.asarray(inputs["mask_logit"], dtype=np.float32)
    lk = float(np.asarray(inputs["log_kappa"], dtype=np.float32).reshape(-1)[0])
    hw = np.asarray(inputs["head_w"], dtype=np.float32)

    xT = np.ascontiguousarray(x.T)  # (D, B)
    lkb = np.full((128, 1), lk, dtype=np.float32)
    selp = np.zeros((128, 2 * RC), dtype=np.float32)
    selp[:, RC] = 1.0

    in_maps = []
    for c in range(NCORES):
        sl = slice(c * RC, (c + 1) * RC)
        in_maps.append(
            {
                "xT": xT,
                "thT": np.ascontiguousarray(th[sl].T),
                "sgT": np.ascontiguousarray(sg[sl].T),
                "mkT": np.ascontiguousarray(mk[sl].T),
                "lkb": lkb,
                "wcol": np.ascontiguousarray(hw.reshape(-1)[sl].reshape(RC, 1)),
                "selp": selp,
            }
        )
    return in_maps


def _run(inputs, **spmd_kwargs):
    nc = _get_nc()
    in_maps = _make_in_maps(inputs)
    res = run_bass_kernel_spmd(nc, in_maps, core_ids=list(range(NCORES)), **spmd_kwargs)
    hb = np.asarray(inputs["head_b"], dtype=np.float32).reshape(-1)[0]
    y = np.sum([r["y"][0] for r in res.results], axis=0, dtype=np.float32) + hb
    return y.astype(np.float32), res


def kernel(**inputs) -> np.ndarray:
    y, _ = _run(inputs)
    return y


# revision 8
# speedup vs baseline: 1.4998x; 1.4998x over previous
"""CornerNet Trainium2 kernel.

Math (reference):
  t     = kappa * tanh(sign_param) * (x - th)        # (B, R, D)
  s     = sigmoid(t); m = sigmoid(mask_logit)
  gated = 1 - m*(1-s) = (1-m) + m*s
  z     = prod_d gated                               # (B, R)
  y     = z @ head_w.T + head_b                      # (B,)

Device formulation (available act funcs: sigmoid/tanh/exp/ln — no softplus):
  a  = kappa * tanh(sign_param)
  s  = sigmoid(a*x - a*th)     (ScalarE per-partition input affine: scale=a,
                                bias=-a*th — no separate VectorE pass needed)
  g  = c + m*s                 ( c = 1-m = sigmoid(-mask_logit) )
  gp = g[d] * g[d+128]         (pair-product folds the two partition halves)
  lz = sum_{d-pairs} ln(gp) ; z = exp(lz) ; y = z @ w + b

Sharding: tensor-parallel over rules; each of 8 cores takes RC=64 rules and
computes a partial (B,) head dot; host sums partials and adds head_b.

Layout per core: D=256 on partitions (two 128-halves), batch on the free axis.
Per rule: ScalarE sigmoid (input affine does t), VectorE tensor_scalar
g = s*m + c, VectorE tensor_tensor pair-product, ScalarE Ln, then the
partition sum goes to TensorE: lhsT is a (128, 64) sliding window into a
constant matrix whose only nonzero column (+1) is positioned so rule r's sum
lands in PSUM partition r (float32r keeps the PE at 1 cyc/row).  Sigmoid and
Ln live in different activation table sets (~2.7us per switch), so rules are
processed in blocks of KBLK: all sigmoids for the block first, then all Lns.
"""

import numpy as np
from contextlib import ExitStack

import concourse.bass as bass
import concourse.bacc as bacc
import concourse.mybir as mybir
import concourse.tile as tile
from concourse.bass_utils import run_bass_kernel_spmd
from bass_rust import add_dep_helper

B, D, R = 2048, 256, 512
NCORES = 8
RC = R // NCORES            # 64 rules per core
KBLK = 8                    # rules per sigmoid/ln block
CH = 512                    # matmul free-dim chunk (one PSUM bank)
F32 = mybir.dt.float32
F32R = mybir.dt.float32r
AF = mybir.ActivationFunctionType
OP = mybir.AluOpType

_cache = {}


def _build(reps=1):
    nc = bacc.Bacc(None)
    xT = nc.dram_tensor("xT", [D, B], F32, kind="ExternalInput")
    thT = nc.dram_tensor("thT", [D, RC], F32, kind="ExternalInput")
    sgT = nc.dram_tensor("sgT", [D, RC], F32, kind="ExternalInput")
    mkT = nc.dram_tensor("mkT", [D, RC], F32, kind="ExternalInput")
    lkb = nc.dram_tensor("lkb", [128, 1], F32, kind="ExternalInput")
    wcol = nc.dram_tensor("wcol", [RC, 1], F32, kind="ExternalInput")
    selp = nc.dram_tensor("selp", [128, 2 * RC], F32R, kind="ExternalInput")
    y = nc.dram_tensor("y", [1, B], F32, kind="ExternalOutput")

    with tile.TileContext(nc) as tc, ExitStack() as ctx:
        const = ctx.enter_context(tc.tile_pool(name="const", bufs=1))
        sp = ctx.enter_context(tc.tile_pool(name="sp", bufs=2))
        gp_ = ctx.enter_context(tc.tile_pool(name="gp_", bufs=2))
        gpp = ctx.enter_context(tc.tile_pool(name="gpp", bufs=KBLK + 1))
        lp = ctx.enter_context(tc.tile_pool(name="lp", bufs=2))
        psum = ctx.enter_context(
            tc.tile_pool(name="psum", bufs=1, space=bass.MemorySpace.PSUM)
        )

        # ---------------- constant loads ----------------
        xt = []
        for h in range(2):
            t_ = const.tile([128, B], F32, tag=f"xt{h}")
            nc.gpsimd.dma_start(t_[:], xT[h * 128 : (h + 1) * 128, :])
            xt.append(t_)

        tht, sgt, mkt = [], [], []
        for name, dram, lst in (("th", thT, tht), ("sg", sgT, sgt), ("mk", mkT, mkt)):
            for h in range(2):
                t_ = const.tile([128, RC], F32, tag=f"{name}{h}")
                nc.gpsimd.dma_start(t_[:], dram[h * 128 : (h + 1) * 128, :])
                lst.append(t_)

        lkt = const.tile([128, 1], F32, tag="lkt")
        nc.gpsimd.dma_start(lkt[:], lkb[:])
        selpt = const.tile([128, 2 * RC], F32R, tag="selpt")
        nc.gpsimd.dma_start(selpt[:], selp[:])
        wct = const.tile([RC, 1], F32, tag="wct")
        nc.gpsimd.dma_start(wct[:], wcol[:])

        # ---------------- parameter prep ----------------
        kap = const.tile([128, 1], F32, tag="kap")
        nc.scalar.activation(kap[:], lkt[:], AF.Exp)
        nkap = const.tile([128, 1], F32, tag="nkap")
        nc.vector.tensor_scalar(nkap[:], kap[:], -1.0, None, OP.mult)

        aa, nb2, mm_, cc_ = [], [], [], []
        for h in range(2):
            tnh = const.tile([128, RC], F32, tag=f"tnh{h}")
            nc.scalar.activation(tnh[:], sgt[h][:], AF.Tanh)
            a_h = const.tile([128, RC], F32, tag=f"a{h}")
            nc.vector.tensor_scalar(a_h[:], tnh[:], kap[:], None, OP.mult)
            na_h = const.tile([128, RC], F32, tag=f"na{h}")
            nc.vector.tensor_scalar(na_h[:], tnh[:], nkap[:], None, OP.mult)
            nb2_h = const.tile([128, RC], F32, tag=f"nb2{h}")
            nc.vector.tensor_mul(nb2_h[:], na_h[:], tht[h][:])
            aa.append(a_h)
            nb2.append(nb2_h)
            m_h = const.tile([128, RC], F32, tag=f"m{h}")
            nc.scalar.activation(m_h[:], mkt[h][:], AF.Sigmoid)
            c_h = const.tile([128, RC], F32, tag=f"c{h}")
            nc.scalar.activation(c_h[:], mkt[h][:], AF.Sigmoid, scale=-1.0)
            mm_.append(m_h)
            cc_.append(c_h)

        # ---------------- main loop ----------------
        lz = psum.tile([RC, B], F32, tag="lz")
        last_ln = None
        for rep in range(reps):
            for blk in range(RC // KBLK):
                gps = []
                sig_insts = []
                for k in range(KBLK):
                    r = blk * KBLK + k
                    s = sp.tile([128, 2 * B], F32, tag="s")
                    for h in range(2):
                        si = nc.scalar.activation(
                            s[:, h * B : (h + 1) * B],
                            xt[h][:],
                            AF.Sigmoid,
                            bias=nb2[h][:, r : r + 1],
                            scale=aa[h][:, r : r + 1],
                        )
                        # keep sigmoid/ln table-set phases contiguous on ACT
                        if last_ln is not None:
                            add_dep_helper(si.ins, last_ln.ins, False,
                                           "act-table phase blocking")
                        sig_insts.append(si)
                    g = gp_.tile([128, 2 * B], F32, tag="g")
                    for h in range(2):
                        nc.vector.tensor_scalar(
                            g[:, h * B : (h + 1) * B],
                            s[:, h * B : (h + 1) * B],
                            mm_[h][:, r : r + 1],
                            cc_[h][:, r : r + 1],
                            OP.mult,
                            OP.add,
                        )
                    gpt = gpp.tile([128, B], F32, tag="gpt")
                    nc.vector.tensor_mul(gpt[:], g[:, 0:B], g[:, B : 2 * B])
                    gps.append(gpt)
                for k in range(KBLK):
                    r = blk * KBLK + k
                    L = lp.tile([128, B], F32R, tag="L")
                    ln_i = nc.scalar.activation(L[:], gps[k][:], AF.Ln)
                    add_dep_helper(ln_i.ins, sig_insts[-1].ins, False,
                                   "act-table phase blocking")
                    last_ln = ln_i
                    lhsp = selpt[:, RC - r : 2 * RC - r]
                    for c in range(B // CH):
                        nc.tensor.matmul(
                            lz[:, c * CH : (c + 1) * CH],
                            lhsp,
                            L[:, c * CH : (c + 1) * CH],
                            start=(r == 0 and rep == 0),
                            stop=(r == RC - 1 and rep == reps - 1),
                        )

        # ---------------- z = exp(lz), head ----------------
        z_sb = const.tile([RC, B], F32, tag="z")
        nc.scalar.activation(z_sb[:], lz[:], AF.Exp)
        yp = psum.tile([1, B], F32, tag="yp")
        for c in range(B // CH):
            nc.tensor.matmul(
                yp[:, c * CH : (c + 1) * CH],
                wct[:],
                z_sb[:, c * CH : (c + 1) * CH],
                start=True,
                stop=True,
            )
        y_sb = const.tile([1, B], F32, tag="ysb")
        nc.vector.tensor_copy(y_sb[:], yp[:])
        nc.sync.dma_start(y[:], y_sb[:])

    nc.compile()
    return nc


def _get_nc(reps=1):
    key = ("nc", reps)
    if key not in _cache:
        _cache[key] = _build(reps)
    return _cache[key]


def _make_in_maps(inputs):
    x = np.ascontiguousarray(inputs["x"], dtype=np.float32)
    th = np.asarray(inputs["th"], dtype=np.float32)
    sg = np.asarray(inputs["sign_param"], dtype=np.float32)
    mk = np.asarray(inputs["mask_logit"], dtype=np.float32)
    lk = float(np.asarray(inputs["log_kappa"], dtype=np.float32).reshape(-1)[0])
    hw = np.asarray(inputs["head_w"], dtype=np.float32)

    xT = np.ascontiguousarray(x.T)  # (D, B)
    lkb = np.full((128, 1), lk, dtype=np.float32)
    selp = np.zeros((128, 2 * RC), dtype=np.float32)
    selp[:, RC] = 1.0

    in_maps = []
    for c in range(NCORES):
        sl = slice(c * RC, (c + 1) * RC)
        in_maps.append(
            {
                "xT": xT,
                "thT": np.ascontiguousarray(th[sl].T),
                "sgT": np.ascontiguousarray(sg[sl].T),
                "mkT": np.ascontiguousarray(mk[sl].T),
                "lkb": lkb,
                "wcol": np.ascontiguousarray(hw.reshape(-1)[sl].reshape(RC, 1)),
                "selp": selp,
            }
        )
    return in_maps


def _run(inputs, **spmd_kwargs):
    nc = _get_nc()
    in_maps = _make_in_maps(inputs)
    res = run_bass_kernel_spmd(nc, in_maps, core_ids=list(range(NCORES)), **spmd_kwargs)
    hb = np.asarray(inputs["head_b"], dtype=np.float32).reshape(-1)[0]
    y = np.sum([r["y"][0] for r in res.results], axis=0, dtype=np.float32) + hb
    return y.astype(np.float32), res


def kernel(**inputs) -> np.ndarray:
    y, _ = _run(inputs)
    return y
